# revision 12
# baseline (speedup 1.0000x reference)
"""Trainium2 Bass kernel for GQA attention (B=2, L=2048, D=3072, H=24, KV=8,
HD=128, causal, half-split RoPE).

Sharding: TP=4 over heads x DP=2 over batch on 8 NeuronCores.
Core c = 4*b + s handles batch b with q-heads 6s..6s+5 and kv-heads 2s,2s+1.
Each core computes a partial o_proj output [L, D]; the host sums the 4 TP
partials per batch (the "all-reduce after o_proj" done on host at gather time).

Per-core device computation (all matmuls bf16 with fp32 PSUM accumulation):
  xT[D,L] (host-pretransposed, bf16)
  Q^T = Wq_s^T x^T  (per head [128,L]) -> RoPE -> qT
  K^T likewise per kv head -> RoPE
  V   = x Wv_s   natural layout [L, 256]
  per head, per 512-wide q-block: S^T[k,q] chunks via PE, exp on ScalarE
  (scale folded into exp), causal mask on diagonal chunks, AV and
  ones-matmul denominators accumulated in PSUM, normalize into O^T bf16,
  then o_proj partial = O^T.T @ Wo_s -> [L, D] fp32.
"""

import math

import numpy as np
import ml_dtypes

import concourse.bass as bass
import concourse.mybir as mybir
import concourse.tile as tile
from concourse import bacc
from concourse.bass_utils import run_bass_kernel_spmd

BF16NP = ml_dtypes.bfloat16

B, L, D = 2, 2048, 3072
H, KV, HD = 24, 8, 128
GROUP = H // KV          # 3
THETA = 500000.0
SCALE = HD ** -0.5
N_CORES = 8
TP = 4                   # tensor-parallel over heads
NQH = H // TP            # 6 q heads per core
NKH = KV // TP           # 2 kv heads per core
QCOLS = NQH * HD         # 768
KCOLS = NKH * HD         # 256
ND = D // 128            # 24 contraction chunks
NLT = L // 128           # 16 l-tiles
NB = L // 512            # 4 q-blocks
BF = mybir.dt.bfloat16
F32 = mybir.dt.float32


def _ls(i, w=512):
    return slice(i * w, (i + 1) * w)


def _rope_tables():
    half = HD // 2
    inv_freq = 1.0 / (THETA ** (np.arange(half, dtype=np.float64) / half))
    ang = np.arange(L, dtype=np.float64)[:, None] * inv_freq[None, :]  # [L, 64]
    cosT = np.cos(ang).T.astype(np.float32)   # [64, L]
    sinT = np.sin(ang).T.astype(np.float32)
    cosF = np.concatenate([cosT, cosT], 0)    # [128, L]
    sinF = np.concatenate([-sinT, sinT], 0)   # rows 0:64 get -sin
    return cosF.astype(BF16NP), sinF.astype(BF16NP)


def _mask_tiles():
    # mask[r, m, c] = 1 if causal-allowed for diagonal chunk offset m:
    # k = 128*j + r, q = 512*b + c, m = j - 4*b; allowed iff c >= 128*m + r
    r = np.arange(128)[:, None, None]
    m = np.arange(4)[None, :, None]
    c = np.arange(512)[None, None, :]
    return (c >= 128 * m + r).astype(BF16NP)  # [128, 4, 512]


def _emit(nc, phases=(1, 2, 3)):
    xT = nc.dram_tensor("xT", [D, L], BF, kind="ExternalInput")
    wqk = nc.dram_tensor("wqk", [D, QCOLS + KCOLS], BF, kind="ExternalInput")
    wv = nc.dram_tensor("wv", [D, KCOLS], BF, kind="ExternalInput")
    wo = nc.dram_tensor("wo", [QCOLS, D], BF, kind="ExternalInput")
    out = nc.dram_tensor("out", [L, D], BF, kind="ExternalOutput")

    cosF, sinF = _rope_tables()
    cosc = nc.inline_tensor(np.ascontiguousarray(cosF), name="cosc")
    sinc = nc.inline_tensor(np.ascontiguousarray(sinF), name="sinc")
    maskc = nc.inline_tensor(np.ascontiguousarray(_mask_tiles()), name="maskc")

    Exp = mybir.ActivationFunctionType.Exp

    with tile.TileContext(nc) as tc:
        with (
            tc.tile_pool(name="persist", bufs=1) as P,
        ):
            cos_sb = P.tile([128, L], BF, tag="cos")
            nc.sync.dma_start(out=cos_sb, in_=cosc.ap())
            sin_sb = P.tile([128, L], BF, tag="sin")
            nc.gpsimd.dma_start(out=sin_sb, in_=sinc.ap())
            ones_sb = P.tile([128, 128], BF, tag="ones")
            nc.vector.memset(ones_sb, 1.0)

            # persistent activations: K^T (rope'd), V natural, O^T
            kT_sb = [
                P.tile([128, L], BF, tag=f"kT{i}", name=f"kT{i}")
                for i in range(NKH)
            ]
            v_sb = P.tile([128, NLT, KCOLS], BF, tag="vsb")
            oT_sb = P.tile([128, NQH, L], BF, tag="oT")

            # ---- quarter-pipelined projections + attention ----
            # Quarter qt: load xt columns [512qt, 512qt+512), project Q/K
            # (rope fused), project V, then run attention q-block b=qt for
            # all 6 heads (its K/V deps cover exactly quarters <= qt).
            with (
                tc.tile_pool(name="xt", bufs=2) as XT,
                tc.tile_pool(name="wres", bufs=1) as WR,
                tc.tile_pool(name="qtq", bufs=2) as QTQ,
                tc.tile_pool(name="ropet", bufs=2) as RT,
                tc.tile_pool(name="p2", bufs=3) as P2,
                tc.tile_pool(name="ps_qk", bufs=2, space="PSUM") as PQ,
                tc.tile_pool(name="ps_v", bufs=1, space="PSUM") as PV,
                tc.tile_pool(name="ps_sc", bufs=2, space="PSUM") as PS,
                tc.tile_pool(name="ps_o", bufs=2, space="PSUM") as PO,
                tc.tile_pool(name="ps_sum", bufs=1, space="PSUM") as PSM,
            ):
                wqk_sb = WR.tile([128, ND, QCOLS + KCOLS], BF, tag="wqksb")
                wqk_r = wqk.ap().rearrange("(dc p) n -> p dc n", p=128)
                wv_sb = WR.tile([128, ND, KCOLS], BF, tag="wvsb")
                wv_r = wv.ap().rearrange("(dc p) n -> p dc n", p=128)
                mask_sb = WR.tile([128, 4, 512], BF, tag="mask")
                xT_r = xT.ap().rearrange("(dc p) l -> p dc l", p=128)

                LQ = 512
                for qt in range(L // LQ):
                    hs = qt * LQ
                    xt_sb = XT.tile([128, ND, LQ], BF, tag="xt")
                    if qt == 0:
                        # d-aligned interleave across the two rings so the
                        # first matmul's operands (xt d0 + wqk d0) land first
                        for d in range(ND):
                            nc.sync.dma_start(
                                out=xt_sb[:, d, :], in_=xT_r[:, d, hs:hs + LQ]
                            )
                            nc.gpsimd.dma_start(
                                out=wqk_sb[:, d, :], in_=wqk_r[:, d, :]
                            )
                        for d in range(ND):
                            eng = (nc.sync, nc.gpsimd)[d % 2]
                            eng.dma_start(out=wv_sb[:, d, :], in_=wv_r[:, d, :])
                        nc.sync.dma_start(out=mask_sb, in_=maskc.ap())
                    else:
                        for d in range(ND):
                            eng = (nc.sync, nc.gpsimd)[d % 2]
                            eng.dma_start(
                                out=xt_sb[:, d, :], in_=xT_r[:, d, hs:hs + LQ]
                            )
                    # Q^T (into quarter-scoped scratch) and K^T, rope fused
                    qTq = QTQ.tile([128, NQH, LQ], BF, tag="qTq")
                    for mi in range(NQH + NKH):
                        ps = PQ.tile([128, 512], F32, tag="psqk")
                        for d in range(ND):
                            nc.tensor.matmul(
                                ps,
                                lhsT=wqk_sb[:, d, mi * 128:(mi + 1) * 128],
                                rhs=xt_sb[:, d, :],
                                start=(d == 0),
                                stop=(d == ND - 1),
                            )
                        qkb = RT.tile([128, 512], BF, tag="qkb")
                        nc.scalar.copy(qkb, ps)
                        rot = RT.tile([128, 512], BF, tag="rot")
                        nc.vector.tensor_copy(out=rot[0:64, :], in_=qkb[64:128, :])
                        nc.vector.tensor_copy(out=rot[64:128, :], in_=qkb[0:64, :])
                        t1 = RT.tile([128, 512], BF, tag="t1")
                        nc.vector.tensor_mul(t1, qkb, cos_sb[:, hs:hs + LQ])
                        nc.vector.tensor_mul(rot, rot, sin_sb[:, hs:hs + LQ])
                        dst = (qTq[:, mi, :] if mi < NQH
                               else kT_sb[mi - NQH][:, hs:hs + LQ])
                        nc.vector.tensor_add(dst, t1, rot)
                    # V projection (natural layout)
                    for lt in range(LQ // 128):
                        glt = qt * (LQ // 128) + lt
                        pv = PV.tile([128, KCOLS], F32, tag="psv")
                        for d in range(ND):
                            nc.tensor.matmul(
                                pv,
                                lhsT=xt_sb[:, d, lt * 128:(lt + 1) * 128],
                                rhs=wv_sb[:, d, :],
                                start=(d == 0),
                                stop=(d == ND - 1),
                            )
                        nc.scalar.copy(v_sb[:, glt, :], pv)

                    if 2 not in phases:
                        continue
                    # attention for q-block b=qt, all heads
                    b = qt
                    nch = 4 * (b + 1)
                    for h in range(NQH):
                        kv = h // GROUP
                        po = PO.tile([128, 512], F32, tag="po")
                        psm = PSM.tile([128, 512], F32, tag="psm")
                        for j in range(nch):
                            sc = PS.tile([128, 512], F32, tag="sc")
                            nc.tensor.matmul(
                                sc,
                                lhsT=kT_sb[kv][:, j * 128:(j + 1) * 128],
                                rhs=qTq[:, h, :],
                                start=True,
                                stop=True,
                            )
                            pt = P2.tile([128, 512], BF, tag="pt")
                            nc.scalar.activation(pt, sc, Exp, scale=SCALE)
                            if j >= 4 * b:
                                nc.vector.tensor_mul(
                                    pt, pt, mask_sb[:, j - 4 * b, :]
                                )
                            nc.tensor.matmul(
                                po,
                                lhsT=v_sb[:, j, kv * 128:(kv + 1) * 128],
                                rhs=pt,
                                start=(j == 0),
                                stop=(j == nch - 1),
                            )
                            nc.tensor.matmul(
                                psm,
                                lhsT=ones_sb,
                                rhs=pt,
                                start=(j == 0),
                                stop=(j == nch - 1),
                            )
                        rc = P2.tile([128, 512], F32, tag="rc", bufs=2)
                        nc.vector.reciprocal(rc, psm)
                        nc.vector.tensor_mul(oT_sb[:, h, _ls(b)], po, rc)

            if 3 not in phases:
                return
            # ---- o_proj partial ----
            with (
                tc.tile_pool(name="wo", bufs=1) as WO,
                tc.tile_pool(name="stage", bufs=4) as SG,
                tc.tile_pool(name="ps_op", bufs=4, space="PSUM") as POP,
            ):
                wo_sb = WO.tile([128, NQH, D], BF, tag="wosb")
                wo_r = wo.ap().rearrange("(c p) n -> p c n", p=128)
                for c in range(NQH):
                    eng = (nc.sync, nc.gpsimd)[c % 2]
                    eng.dma_start(out=wo_sb[:, c, :], in_=wo_r[:, c, :])
                out_r = out.ap().rearrange(
                    "(lt p) (et n) -> p lt et n", p=128, n=512
                )
                for lt in range(NLT):
                    for e in range(D // 512):
                        pp = POP.tile([128, 512], F32, tag="pp")
                        for c in range(NQH):
                            nc.tensor.matmul(
                                pp,
                                lhsT=oT_sb[:, c, lt * 128:(lt + 1) * 128],
                                rhs=wo_sb[:, c, _ls(e)],
                                start=(c == 0),
                                stop=(c == NQH - 1),
                            )
                        st = SG.tile([128, 512], BF, tag="st")
                        if e % 2 == 0:
                            nc.vector.tensor_copy(st, pp)
                        else:
                            nc.scalar.copy(st, pp)
                        oeng = (nc.sync, nc.scalar, nc.gpsimd)[(lt * 6 + e) % 3]
                        oeng.dma_start(out=out_r[:, lt, e, :], in_=st)
    return nc


_NC_CACHE = {}


def build(phases=(1, 2, 3)):
    key = tuple(phases)
    if key not in _NC_CACHE:
        nc = bacc.Bacc(
            "TRN2", target_bir_lowering=False, debug=False, num_devices=N_CORES
        )
        _emit(nc, phases)
        nc.compile()
        _NC_CACHE[key] = nc
    return _NC_CACHE[key]


def prep_in_maps(x, Wq, Wk, Wv, Wo):
    """Shard + cast + layout the full inputs into 8 per-core input maps."""
    x = np.asarray(x)
    Wq, Wk, Wv, Wo = (np.asarray(a) for a in (Wq, Wk, Wv, Wo))
    in_maps = []
    wqk_s = [
        np.ascontiguousarray(np.hstack([
            Wq[:, s * QCOLS:(s + 1) * QCOLS],
            Wk[:, s * KCOLS:(s + 1) * KCOLS],
        ])).astype(BF16NP)
        for s in range(TP)
    ]
    wv_s = [np.ascontiguousarray(Wv[:, s * KCOLS:(s + 1) * KCOLS]).astype(BF16NP)
            for s in range(TP)]
    wo_s = [np.ascontiguousarray(Wo[s * QCOLS:(s + 1) * QCOLS, :]).astype(BF16NP)
            for s in range(TP)]
    xT_b = [np.ascontiguousarray(x[b].T).astype(BF16NP) for b in range(B)]
    for core in range(N_CORES):
        b, s = divmod(core, TP)
        in_maps.append({
            "xT": xT_b[b],
            "wqk": wqk_s[s],
            "wv": wv_s[s],
            "wo": wo_s[s],
        })
    return in_maps


def kernel(x, Wq, Wk, Wv, Wo):
    nc = build()
    in_maps = prep_in_maps(x, Wq, Wk, Wv, Wo)
    res = run_bass_kernel_spmd(nc, in_maps, list(range(N_CORES)))
    out = np.zeros((B, L, D), np.float32)
    for core in range(N_CORES):
        b, _s = divmod(core, TP)
        out[b] += res.results[core]["out"].astype(np.float32)
    return out
